# revision 34
# baseline (speedup 1.0000x reference)
"""Single-head attention kernel for Trainium2, 8 NeuronCores.

Problem: x[4, 4096, 1024] f32; Wq/Wk/Wv [1024, 64]; bq/bk/bv [64].
  Q/K/V = x @ W + b ; out = softmax(Q K^T / 8) @ V  -> [4, 4096, 64]

Sharding: 8 shards = (batch b, query-half h). Each core computes K/V for
all 4096 tokens of its batch and attention for its 2048 queries.

v8 design (engine balance: ACT exp ~72us busy, PE ~75us busy):
  - Bias algebra: K-bias dropped (softmax invariant to per-query consts),
    V-bias added on the host (attn rows sum to 1). Only bq on device.
  - ONE uniform phase, no PSUM pool transition. Banks: score tiles
    2x[128,1024] (4) + kvp (1) + t2 (1) + oq (1) + den (1) = 8.
  - Split-pack projections: each 512-token chunk packs tokens 0:256
    with [Wk|Wv] (K on rows 0:64) and 256:512 with [Wv|Wk] (K on rows
    64:128), so a row-tiled score PAIR forms within ONE chunk - the
    first exp fires ~7us earlier (chunk 0 alone, not chunks 0+1).
    [Wv|Wk] is derived on-device from [Wk|Wv] by a DVE column swap.
  - Scores: row-tiled pairs -> S^T [128 keys, 1024 q] PSUM tiles; ACT
    exp (scale 1/8) -> bf16 P tiles; 64 uniform N=1024 slots.
  - AV numerator: col-tiled M=64 pairs (even arrivals -> psum[0:64],
    odd -> psum[64:128]; host adds). Denominator: 4x col-tiled M=1
    quads (ones lhsT) -> partitions 0/32/64/96 of the den bank; host
    sums. Drains only touch halves >=2 ACT slots old so pair/quad
    members are ready together and pop back-to-back (2x/4x).
  - Q projections run right after their own chunk, freeing the oq bank
    early so numerator drains span the whole kernel.
  - Output per qs: numerator [128, 512] bf16 (one DMA) + denominator
    partials (4 row DMAs); host reduces, divides, transposes, adds bv.
"""

from contextlib import ExitStack

import ml_dtypes
import numpy as np

import concourse.bass as bass
import concourse.mybir as mybir
from concourse import bacc
import concourse.tile as tile
from concourse.bass_utils import run_bass_kernel_spmd

B = 4
S = 4096
D = 1024
H = 64
NCORES = 8
TQ = S // 2      # queries per core
CH = 512         # token chunk for projections
HC = CH // 2     # split-pack half chunk
QS = 512         # query slice for attention
NKT = D // 128   # 8 contraction tiles for projections
NCH = S // CH    # 8 token chunks
NK2 = S // 128   # 32 key tiles for attention
NQS = TQ // QS   # 4 query slices
NPAIR = NK2 // 2  # 16 row-tiled score pairs per query slice
SCALE = 1.0 / 8.0  # 1/sqrt(64)

F32 = mybir.dt.float32
F32R = mybir.dt.float32r
BF16 = mybir.dt.bfloat16


def k2_of_slot(half, p):
    """Global key-tile index for pair p's lo/hi slot.

    Pair p lives in chunk p//2: lo = tokens (p%2)*128, hi = 256+(p%2)*128.
    """
    return 4 * (p // 2) + (2 if half else 0) + p % 2


def build_nc():
    nc = bacc.Bacc(None, target_bir_lowering=False)
    xH = nc.dram_tensor("xH", [128, NCH, NKT, CH], BF16, kind="ExternalInput")
    wkv_e = nc.dram_tensor("wkv_e", [128, NKT, 128], BF16, kind="ExternalInput")
    wq2 = nc.dram_tensor("wq2", [128, NKT, 128], BF16, kind="ExternalInput")
    biasd = nc.dram_tensor("biasd", [128, 1], F32, kind="ExternalInput")
    identd = nc.dram_tensor("identd", [128, 128], F32R, kind="ExternalInput")
    outn = nc.dram_tensor("outn", [NQS, 128, QS], BF16, kind="ExternalOutput")
    outd = nc.dram_tensor("outd", [NQS, 4, QS], F32, kind="ExternalOutput")

    with ExitStack() as ctx:
        tc = ctx.enter_context(tile.TileContext(nc))
        singles = ctx.enter_context(tc.tile_pool(name="singles", bufs=1))
        persist = ctx.enter_context(tc.tile_pool(name="persist", bufs=1))

        KT = persist.tile([128, NPAIR * 128], BF16)
        QT2 = persist.tile([128, TQ], BF16)     # Q^T duplicated on both halves
        Vaug = persist.tile([128, NK2, 64], BF16)  # V natural layout

        with (
            tc.tile_pool(name="xt", bufs=3) as xt_pool,
            tc.tile_pool(name="vt", bufs=8) as vt_pool,
            tc.tile_pool(name="p", bufs=34) as p_pool,
            tc.tile_pool(name="osb", bufs=2) as osb_pool,
            tc.tile_pool(name="osbd", bufs=2) as osbd_pool,
            tc.tile_pool(name="stps", bufs=2, space="PSUM") as st_pool,
            tc.tile_pool(name="kvpe", bufs=1, space="PSUM") as kv_e_pool,
            tc.tile_pool(name="kvpo", bufs=1, space="PSUM") as kv_o_pool,
            tc.tile_pool(name="oqps", bufs=1, space="PSUM") as oq_ps_pool,
            tc.tile_pool(name="dnps", bufs=1, space="PSUM") as dn_ps_pool,
        ):
            kvst = {}  # c -> [xtc, kvp, vt, (qp)] chunk state

            # qs -> [(k2, view512, slot_id)]
            arrivals = {q: [] for q in range(NQS)}
            slot_seq = [0]
            num_cur = [0 for _ in range(NQS)]
            den_cur = [0 for _ in range(NQS)]
            num_ptr = [0]
            den_ptr = [0]
            num_st = {}
            den_st = {}
            n_half = [0]
            n_num = [0]
            n_den = [0]
            kvc_done = [0]  # chunks whose V tiles are in Vaug

            def emit_st(qs, p):
                st = st_pool.tile([128, 2 * QS], F32, name="st")
                for i, half in enumerate((0, 1)):
                    rows = slice(64, 128) if half else slice(0, 64)
                    nc.tensor.matmul(
                        st[:, i * QS : (i + 1) * QS],
                        KT[rows, p * 128 : (p + 1) * 128],
                        QT2[rows, qs * QS : (qs + 1) * QS],
                        start=True,
                        stop=True,
                    )
                p_tile = p_pool.tile([128, 2 * QS], BF16, name="pt")
                nc.scalar.activation(
                    p_tile, st, mybir.ActivationFunctionType.Exp, scale=SCALE
                )
                for i, half in enumerate((0, 1)):
                    arrivals[qs].append(
                        (k2_of_slot(half, p),
                         p_tile[:, i * QS : (i + 1) * QS],
                         slot_seq[0])
                    )
                slot_seq[0] += 1
                n_half[0] += 2

            def n_ready(qs, cur, cutoff, vaug=False):
                n = 0
                for e in arrivals[qs][cur:]:
                    if e[2] > cutoff:
                        break
                    if vaug and e[0] // 4 >= kvc_done[0]:
                        break
                    n += 1
                return n

            def emit_num(cutoff):
                qs = num_ptr[0]
                if qs >= NQS:
                    return False
                avail = n_ready(qs, num_cur[qs], cutoff, vaug=True)
                last = num_cur[qs] + avail >= NK2
                if avail < 2 and not (avail == 1 and last):
                    return False
                if qs not in num_st:
                    num_st[qs] = [
                        oq_ps_pool.tile([128, QS], F32, name="op", tag="oq"),
                        0,
                        0,
                    ]
                st = num_st[qs]
                take = min(2, avail)
                for j in range(take):
                    k2, view, _ = arrivals[qs][num_cur[qs]]
                    num_cur[qs] += 1
                    grp = j if take == 2 else (0 if st[1] < NK2 // 2 else 1)
                    n = st[1 + grp]
                    rows = slice(0, 64) if grp == 0 else slice(64, 128)
                    nc.tensor.matmul(
                        st[0][rows, :],
                        Vaug[:, k2, :],
                        view,
                        start=(n == 0),
                        stop=(n == NK2 // 2 - 1),
                    )
                    st[1 + grp] += 1
                    n_num[0] += 1
                if num_cur[qs] == NK2:
                    osb = osb_pool.tile([128, QS], BF16, name="osb")
                    nc.vector.tensor_copy(osb, st[0])
                    nc.sync.dma_start(outn[qs, :, :], osb)
                    del num_st[qs]
                    num_ptr[0] += 1
                return True

            def emit_den(cutoff):
                qs = den_ptr[0]
                if qs >= NQS:
                    return False
                avail = n_ready(qs, den_cur[qs], cutoff)
                if avail < 4:
                    return False
                if qs not in den_st:
                    den_st[qs] = [
                        dn_ps_pool.tile([128, QS], F32, name="dn", tag="dn"),
                        [0, 0, 0, 0],
                    ]
                st = den_st[qs]
                for j in range(4):
                    _, view, _ = arrivals[qs][den_cur[qs]]
                    den_cur[qs] += 1
                    n = st[1][j]
                    nc.tensor.matmul(
                        st[0][32 * j : 32 * j + 1, :],
                        ones1[:, 0:1],
                        view,
                        start=(n == 0),
                        stop=(n == NK2 // 4 - 1),
                        tile_position=(0, 32 * j),
                    )
                    st[1][j] += 1
                    n_den[0] += 1
                if den_cur[qs] == NK2:
                    osbd = osbd_pool.tile([97, QS], F32, name="osbd")
                    nc.vector.tensor_copy(osbd, st[0][0:97, :])
                    for i in range(4):
                        nc.sync.dma_start(
                            outd[qs, i : i + 1, :],
                            osbd[32 * i : 32 * i + 1, :],
                        )
                    del den_st[qs]
                    den_ptr[0] += 1
                return True

            # Warmup: preload the exp ACT table and run matmuls with no
            # readers so the PE HAM un-throttles during the DMA head.
            wrm = singles.tile([128, QS], BF16)
            nc.vector.memset(wrm, 0.0)
            wrm2 = singles.tile([128, 32], BF16)
            nc.scalar.activation(wrm2, wrm[:, 0:32],
                                 mybir.ActivationFunctionType.Exp,
                                 scale=SCALE)
            for _ in range(12):
                wps = st_pool.tile([128, 2 * QS], F32, name="st")
                nc.tensor.matmul(wps[:, 0:QS], wrm[:, 0:128], wrm,
                                 start=True, stop=True)
            ones1 = singles.tile([128, 1], BF16)
            nc.vector.memset(ones1, 1.0)

            # DMA issue order = queue-FIFO transfer order: wkv_e, c0(A,B),
            # wq2+bias, c1(A,B), ident, c2, ...  [Wv|Wk] is DVE-derived.
            wkv_e_sb = singles.tile([128, NKT, 128], BF16)
            nc.sync.dma_start(wkv_e_sb, wkv_e[:, :, :])
            wkv_o_sb = singles.tile([128, NKT, 128], BF16)
            nc.vector.tensor_copy(wkv_o_sb[:, :, 0:64], wkv_e_sb[:, :, 64:128])
            nc.vector.tensor_copy(wkv_o_sb[:, :, 64:128], wkv_e_sb[:, :, 0:64])
            bias_sb = singles.tile([128, 1], F32)
            wq2_sb = singles.tile([128, NKT, 128], BF16)
            ident = singles.tile([128, 128], F32R)

            def kv_a(c):
                xtc = xt_pool.tile([128, NKT, CH], BF16, name="xtc")
                nc.sync.dma_start(xtc[:, 0:4, :], xH[:, c, 0:4, :])
                nc.sync.dma_start(xtc[:, 4:8, :], xH[:, c, 4:8, :])
                if c == 0:
                    nc.sync.dma_start(bias_sb, biasd[:, :])
                    nc.sync.dma_start(wq2_sb, wq2[:, :, :])
                if c == 1:
                    nc.sync.dma_start(ident, identd[:, :])
                kve = kv_e_pool.tile([128, HC], F32, name="kve")
                kvo = kv_o_pool.tile([128, HC], F32, name="kvo")
                kvst[c] = [xtc, (kve, kvo), None]
                for kt in range(4):
                    nc.tensor.matmul(
                        kve,
                        wkv_e_sb[:, kt, :],
                        xtc[:, kt, 0:HC],
                        start=(kt == 0),
                        stop=False,
                    )
                    nc.tensor.matmul(
                        kvo,
                        wkv_o_sb[:, kt, :],
                        xtc[:, kt, HC:CH],
                        start=(kt == 0),
                        stop=False,
                    )

            def kv_b(c):
                xtc, (kve, kvo), _ = kvst[c]
                for kt in range(4, NKT):
                    nc.tensor.matmul(
                        kve,
                        wkv_e_sb[:, kt, :],
                        xtc[:, kt, 0:HC],
                        start=False,
                        stop=(kt == NKT - 1),
                    )
                    nc.tensor.matmul(
                        kvo,
                        wkv_o_sb[:, kt, :],
                        xtc[:, kt, HC:CH],
                        start=False,
                        stop=(kt == NKT - 1),
                    )
                # tokens 0:256: K on rows 0:64, V on 64:128 (even pack);
                # tokens 256:512: K on 64:128, V on 0:64 (odd pack).
                nc.vector.tensor_copy(
                    KT[0:64, (2 * c) * 128 : (2 * c + 2) * 128],
                    kve[0:64, :],
                )
                nc.vector.tensor_copy(
                    KT[64:128, (2 * c) * 128 : (2 * c + 2) * 128],
                    kvo[64:128, :],
                )
                vt = vt_pool.tile([128, CH], F32R, name="vt")
                nc.vector.tensor_copy(vt[64:128, 0:HC], kve[64:128, :])
                nc.vector.tensor_copy(vt[0:64, HC:CH], kvo[0:64, :])
                kvst[c][2] = vt

            def q_a(c):
                xtc = kvst[c][0]
                qp = oq_ps_pool.tile([128, CH], F32, name="qp", tag="oq")
                kvst[c].append(qp)
                for kt in range(4):
                    nc.tensor.matmul(
                        qp,
                        wq2_sb[:, kt, :],
                        xtc[:, kt, :],
                        start=(kt == 0),
                        stop=False,
                    )

            def q_b(c):
                xtc, _, _, qp = kvst[c]
                for kt in range(4, NKT):
                    nc.tensor.matmul(
                        qp,
                        wq2_sb[:, kt, :],
                        xtc[:, kt, :],
                        start=False,
                        stop=(kt == NKT - 1),
                    )
                nc.vector.tensor_scalar_add(
                    QT2[:, c * CH : (c + 1) * CH], qp, bias_sb[:, 0:1]
                )

            def kv_c(c):
                vt = kvst[c][2]
                for s4 in range(CH // 128):
                    vrows = slice(64, 128) if s4 < 2 else slice(0, 64)
                    t2 = dn_ps_pool.tile([128, 128], F32, name="t2", tag="dn")
                    nc.tensor.transpose(
                        t2[:, 0:64].bitcast(F32R),
                        vt[vrows, s4 * 128 : (s4 + 1) * 128],
                        ident[vrows, vrows],
                    )
                    nc.vector.tensor_copy(
                        Vaug[:, c * (CH // 128) + s4, :], t2[:, 0:64]
                    )
                kvc_done[0] = c + 1

            # ---- single fused phase ----
            kv_a(0); kv_b(0); q_a(0); q_b(0)

            parts = [
                ("kva", 1), ("kvb", 1), ("qa", 1), ("qb", 1), ("kvc", 0),
                ("kva", 2), ("kvb", 2), ("qa", 2), ("qb", 2), ("kvc", 1),
                ("kva", 3), ("kvb", 3), ("qa", 3), ("qb", 3), ("kvc", 2),
                ("kva", 4), ("kvb", 4), ("kvc", 3),
                ("kva", 5), ("kvb", 5), ("kvc", 4),
                ("kva", 6), ("kvb", 6), ("kvc", 5),
                ("kva", 7), ("kvb", 7), ("kvc", 6), ("kvc", 7),
            ]
            part_fn = {"kva": kv_a, "kvb": kv_b, "qa": q_a, "qb": q_b,
                       "kvc": kv_c}
            part_want = {"kva": 1, "kvb": 1, "qa": 1, "qb": 1, "kvc": 2}
            kvb_done = 0
            qb_done = [True, False, False, False]
            next_pair = [0, 0, 0, 0]

            def slots_avail(qs):
                if not qb_done[qs]:
                    return False
                p = next_pair[qs]
                return p < NPAIR and p // 2 <= kvb_done

            def take_slot(qs):
                p = next_pair[qs]
                next_pair[qs] += 1
                emit_st(qs, p)

            num_ok = [False]
            den_ok = [False]

            def drains(floor, dcap=1, ncap=2):
                cutoff = slot_seq[0] - 3
                if den_ok[0]:
                    k = 0
                    while k < dcap and emit_den(cutoff):
                        k += 1
                if num_ok[0]:
                    k = 0
                    while k < ncap and (n_half[0] - n_num[0]) > floor and \
                            emit_num(cutoff):
                        k += 1

            for kind, c in parts:
                want = part_want[kind]
                for qs in range(NQS):
                    if want and slots_avail(qs):
                        take_slot(qs)
                        want -= 1
                part_fn[kind](c)
                if kind == "kvb":
                    kvb_done = c
                if kind == "qb":
                    qb_done[c] = True
                    if c == 3:
                        num_ok[0] = True
                drains(14)

            # ---- post-parts: remaining slots + decaying drain floor ----
            remaining = [(qs, p) for qs in range(NQS)
                         for p in range(next_pair[qs], NPAIR)]
            n_rem = len(remaining)
            num_ok[0] = True
            den_ok[0] = True
            backlog0 = n_half[0] - n_num[0]
            for j, (qs, p) in enumerate(remaining):
                emit_st(qs, p)
                floor = max(2, (backlog0 * (n_rem - 1 - j)) // n_rem)
                drains(floor, dcap=2, ncap=3)
            inf = 1 << 30
            while emit_num(inf):
                pass
            while emit_den(inf):
                pass
            assert n_num[0] == NQS * NK2, n_num[0]
            assert n_den[0] == NQS * NK2, n_den[0]
    return nc


_NC_CACHE = None


def _get_nc():
    global _NC_CACHE
    if _NC_CACHE is None:
        nc = build_nc()
        nc.finalize()
        _NC_CACHE = nc
    return _NC_CACHE


LAST_RESULT = None
RUN_KWARGS = {}


def kernel(x, Wq, bq, Wk, bk, Wv, bv):
    global LAST_RESULT
    x = np.asarray(x, dtype=np.float32)
    Wq = np.asarray(Wq, dtype=np.float32)
    Wk = np.asarray(Wk, dtype=np.float32)
    Wv = np.asarray(Wv, dtype=np.float32)
    bq_a = np.asarray(bq, dtype=np.float32)
    bv_a = np.asarray(bv, dtype=np.float32)

    bf = ml_dtypes.bfloat16

    # per 128-row contraction tile [128, kt, 128]: [Wk|Wv]
    def pack2(wa, wb):
        h = np.empty((128, NKT, 128), np.float32)
        h[:, :, :64] = wa.reshape(NKT, 128, 64).transpose(1, 0, 2)
        h[:, :, 64:] = wb.reshape(NKT, 128, 64).transpose(1, 0, 2)
        return np.ascontiguousarray(h.reshape(128, NKT * 128)).astype(bf)

    wkv_e_host = pack2(Wk, Wv).reshape(128, NKT, 128)
    wq2_host = pack2(Wq, Wq).reshape(128, NKT, 128)
    bias_host = np.zeros((128, 1), np.float32)
    bias_host[:, 0] = np.concatenate([bq_a, bq_a])
    ident_host = np.eye(128, dtype=np.float32)

    in_maps = []
    for c in range(NCORES):
        b, h = divmod(c, 2)
        xb = x[b]
        if h == 1:
            xb = np.concatenate([xb[TQ:], xb[:TQ]], axis=0)
        # xH[p, c, k, t] = x^T[k*128+p, c*512+t]
        xh = np.ascontiguousarray(
            xb.T.astype(bf).reshape(NKT, 128, NCH, CH).transpose(1, 2, 0, 3)
        ).reshape(128, NCH, NKT, CH)
        in_maps.append(
            {
                "xH": xh,
                "wkv_e": wkv_e_host,
                "wq2": wq2_host,
                "biasd": bias_host,
                "identd": ident_host,
            }
        )

    nc = _get_nc()
    res = run_bass_kernel_spmd(nc, in_maps, core_ids=list(range(NCORES)), **RUN_KWARGS)
    LAST_RESULT = res

    outp = np.empty((B, S, H), np.float32)
    for c in range(NCORES):
        b, h = divmod(c, 2)
        on = np.asarray(res.results[c]["outn"], np.float32)  # [qs, 128, 512]
        num = on[:, 0:64, :] + on[:, 64:128, :]     # [qs, h, q]
        den = res.results[c]["outd"].sum(axis=1, keepdims=True)
        outp[b, h * TQ : (h + 1) * TQ] = (
            (num / den).transpose(0, 2, 1).reshape(TQ, H) + bv_a
        )
    return outp


# revision 36
# speedup vs baseline: 1.0588x; 1.0588x over previous
"""Single-head attention kernel for Trainium2, 8 NeuronCores.

Problem: x[4, 4096, 1024] f32; Wq/Wk/Wv [1024, 64]; bq/bk/bv [64].
  Q/K/V = x @ W + b ; out = softmax(Q K^T / 8) @ V  -> [4, 4096, 64]

Sharding: 8 shards = (batch b, query-half h). Each core computes K/V for
all 4096 tokens of its batch and attention for its 2048 queries.

v8 design (engine balance: ACT exp ~72us busy, PE ~75us busy):
  - Bias algebra: K-bias dropped (softmax invariant to per-query consts),
    V-bias added on the host (attn rows sum to 1). Only bq on device.
  - ONE uniform phase, no PSUM pool transition. Banks: score tiles
    2x[128,1024] (4) + kvp (1) + t2 (1) + oq (1) + den (1) = 8.
  - Split-pack projections: each 512-token chunk packs tokens 0:256
    with [Wk|Wv] (K on rows 0:64) and 256:512 with [Wv|Wk] (K on rows
    64:128), so a row-tiled score PAIR forms within ONE chunk - the
    first exp fires ~7us earlier (chunk 0 alone, not chunks 0+1).
    [Wv|Wk] is derived on-device from [Wk|Wv] by a DVE column swap.
  - Scores: row-tiled pairs -> S^T [128 keys, 1024 q] PSUM tiles; ACT
    exp (scale 1/8) -> bf16 P tiles; 64 uniform N=1024 slots.
  - AV numerator: col-tiled M=64 pairs (even arrivals -> psum[0:64],
    odd -> psum[64:128]; host adds). Denominator: 4x col-tiled M=1
    quads (ones lhsT) -> partitions 0/32/64/96 of the den bank; host
    sums. Drains only touch halves >=2 ACT slots old so pair/quad
    members are ready together and pop back-to-back (2x/4x).
  - Q projections run right after their own chunk, freeing the oq bank
    early so numerator drains span the whole kernel.
  - Output per qs: numerator [128, 512] bf16 (one DMA) + denominator
    partials (4 row DMAs); host reduces, divides, transposes, adds bv.
"""

from contextlib import ExitStack

import ml_dtypes
import numpy as np

import concourse.bass as bass
import concourse.mybir as mybir
from concourse import bacc
import concourse.tile as tile
from concourse.bass_utils import run_bass_kernel_spmd

B = 4
S = 4096
D = 1024
H = 64
NCORES = 8
TQ = S // 2      # queries per core
CH = 512         # token chunk for projections
HC = CH // 2     # split-pack half chunk
QS = 512         # query slice for attention
NKT = D // 128   # 8 contraction tiles for projections
NCH = S // CH    # 8 token chunks
NK2 = S // 128   # 32 key tiles for attention
NQS = TQ // QS   # 4 query slices
NPAIR = NK2 // 2  # 16 row-tiled score pairs per query slice
SCALE = 1.0 / 8.0  # 1/sqrt(64)

F32 = mybir.dt.float32
F32R = mybir.dt.float32r
BF16 = mybir.dt.bfloat16


def k2_of_slot(half, p):
    """Global key-tile index for pair p's lo/hi slot.

    Pair p lives in chunk p//2: lo = tokens (p%2)*128, hi = 256+(p%2)*128.
    """
    return 4 * (p // 2) + (2 if half else 0) + p % 2


def build_nc():
    nc = bacc.Bacc(None, target_bir_lowering=False)
    xH = nc.dram_tensor("xH", [128, NCH, NKT, CH], BF16, kind="ExternalInput")
    wkv_e = nc.dram_tensor("wkv_e", [128, NKT, 128], BF16, kind="ExternalInput")
    wq2 = nc.dram_tensor("wq2", [128, NKT, 128], BF16, kind="ExternalInput")
    biasd = nc.dram_tensor("biasd", [128, 1], F32, kind="ExternalInput")
    identd = nc.dram_tensor("identd", [128, 128], F32R, kind="ExternalInput")
    outn = nc.dram_tensor("outn", [NQS, 128, QS], BF16, kind="ExternalOutput")
    outd = nc.dram_tensor("outd", [NQS, 4, QS], F32, kind="ExternalOutput")

    with ExitStack() as ctx:
        tc = ctx.enter_context(tile.TileContext(nc))
        singles = ctx.enter_context(tc.tile_pool(name="singles", bufs=1))
        persist = ctx.enter_context(tc.tile_pool(name="persist", bufs=1))

        KT = persist.tile([128, NPAIR * 128], BF16)
        QT2 = persist.tile([128, TQ], BF16)     # Q^T duplicated on both halves
        Vaug = persist.tile([128, NK2, 64], BF16)  # V natural layout

        with (
            tc.tile_pool(name="xt", bufs=3) as xt_pool,
            tc.tile_pool(name="vt", bufs=8) as vt_pool,
            tc.tile_pool(name="p", bufs=34) as p_pool,
            tc.tile_pool(name="osb", bufs=2) as osb_pool,
            tc.tile_pool(name="osbd", bufs=2) as osbd_pool,
            tc.tile_pool(name="stps", bufs=2, space="PSUM") as st_pool,
            tc.tile_pool(name="kvpe", bufs=1, space="PSUM") as kv_e_pool,
            tc.tile_pool(name="kvpo", bufs=1, space="PSUM") as kv_o_pool,
            tc.tile_pool(name="oqps", bufs=1, space="PSUM") as oq_ps_pool,
            tc.tile_pool(name="dnps", bufs=1, space="PSUM") as dn_ps_pool,
        ):
            kvst = {}  # c -> [xtc, kvp, vt, (qp)] chunk state

            # qs -> [(k2, view512, slot_id)]
            arrivals = {q: [] for q in range(NQS)}
            slot_seq = [0]
            num_cur = [0 for _ in range(NQS)]
            den_cur = [0 for _ in range(NQS)]
            num_ptr = [0]
            den_ptr = [0]
            num_st = {}
            den_st = {}
            n_half = [0]
            n_num = [0]
            n_den = [0]
            kvc_done = [0]  # chunks whose V tiles are in Vaug

            def emit_st(qs, p):
                st = st_pool.tile([128, 2 * QS], F32, name="st")
                for i, half in enumerate((0, 1)):
                    rows = slice(64, 128) if half else slice(0, 64)
                    nc.tensor.matmul(
                        st[:, i * QS : (i + 1) * QS],
                        KT[rows, p * 128 : (p + 1) * 128],
                        QT2[rows, qs * QS : (qs + 1) * QS],
                        start=True,
                        stop=True,
                    )
                p_tile = p_pool.tile([128, 2 * QS], BF16, name="pt")
                nc.scalar.activation(
                    p_tile, st, mybir.ActivationFunctionType.Exp, scale=SCALE
                )
                for i, half in enumerate((0, 1)):
                    arrivals[qs].append(
                        (k2_of_slot(half, p),
                         p_tile[:, i * QS : (i + 1) * QS],
                         slot_seq[0])
                    )
                slot_seq[0] += 1
                n_half[0] += 2

            def n_ready(qs, cur, cutoff, vaug=False):
                n = 0
                for e in arrivals[qs][cur:]:
                    if e[2] > cutoff:
                        break
                    if vaug and e[0] // 4 >= kvc_done[0]:
                        break
                    n += 1
                return n

            def emit_num(cutoff):
                qs = num_ptr[0]
                if qs >= NQS:
                    return False
                avail = n_ready(qs, num_cur[qs], cutoff, vaug=True)
                last = num_cur[qs] + avail >= NK2
                if avail < 2 and not (avail == 1 and last):
                    return False
                if qs not in num_st:
                    num_st[qs] = [
                        oq_ps_pool.tile([128, QS], F32, name="op", tag="oq"),
                        0,
                        0,
                    ]
                st = num_st[qs]
                take = min(2, avail)
                for j in range(take):
                    k2, view, _ = arrivals[qs][num_cur[qs]]
                    num_cur[qs] += 1
                    grp = j if take == 2 else (0 if st[1] < NK2 // 2 else 1)
                    n = st[1 + grp]
                    rows = slice(0, 64) if grp == 0 else slice(64, 128)
                    nc.tensor.matmul(
                        st[0][rows, :],
                        Vaug[:, k2, :],
                        view,
                        start=(n == 0),
                        stop=(n == NK2 // 2 - 1),
                    )
                    st[1 + grp] += 1
                    n_num[0] += 1
                if num_cur[qs] == NK2:
                    osb = osb_pool.tile([128, QS], BF16, name="osb")
                    nc.vector.tensor_copy(osb, st[0])
                    nc.sync.dma_start(outn[qs, :, :], osb)
                    del num_st[qs]
                    num_ptr[0] += 1
                return True

            def emit_den(cutoff):
                qs = den_ptr[0]
                if qs >= NQS:
                    return False
                avail = n_ready(qs, den_cur[qs], cutoff)
                if avail < 4:
                    return False
                if qs not in den_st:
                    den_st[qs] = [
                        dn_ps_pool.tile([128, QS], F32, name="dn", tag="dn"),
                        [0, 0, 0, 0],
                    ]
                st = den_st[qs]
                for j in range(4):
                    _, view, _ = arrivals[qs][den_cur[qs]]
                    den_cur[qs] += 1
                    n = st[1][j]
                    nc.tensor.matmul(
                        st[0][32 * j : 32 * j + 1, :],
                        ones1[:, 0:1],
                        view,
                        start=(n == 0),
                        stop=(n == NK2 // 4 - 1),
                        tile_position=(0, 32 * j),
                    )
                    st[1][j] += 1
                    n_den[0] += 1
                if den_cur[qs] == NK2:
                    osbd = osbd_pool.tile([97, QS], F32, name="osbd")
                    nc.vector.tensor_copy(osbd, st[0][0:97, :])
                    for i in range(4):
                        nc.sync.dma_start(
                            outd[qs, i : i + 1, :],
                            osbd[32 * i : 32 * i + 1, :],
                        )
                    del den_st[qs]
                    den_ptr[0] += 1
                return True

            # Warmup: preload the exp ACT table and run matmuls with no
            # readers so the PE HAM un-throttles during the DMA head.
            wrm = singles.tile([128, QS], BF16)
            nc.vector.memset(wrm, 0.0)
            wrm2 = singles.tile([128, 32], BF16)
            nc.scalar.activation(wrm2, wrm[:, 0:32],
                                 mybir.ActivationFunctionType.Exp,
                                 scale=SCALE)
            for _ in range(18):
                wps = st_pool.tile([128, 2 * QS], F32, name="st")
                nc.tensor.matmul(wps[:, 0:QS], wrm[:, 0:128], wrm,
                                 start=True, stop=True)
            ones1 = singles.tile([128, 1], BF16)
            nc.vector.memset(ones1, 1.0)

            # DMA issue order = queue-FIFO transfer order: wkv_e, c0(A,B),
            # wq2+bias, c1(A,B), ident, c2, ...  [Wv|Wk] is DVE-derived.
            wkv_e_sb = singles.tile([128, NKT, 128], BF16)
            nc.sync.dma_start(wkv_e_sb, wkv_e[:, :, :])
            wkv_o_sb = singles.tile([128, NKT, 128], BF16)
            nc.vector.tensor_copy(wkv_o_sb[:, :, 0:64], wkv_e_sb[:, :, 64:128])
            nc.vector.tensor_copy(wkv_o_sb[:, :, 64:128], wkv_e_sb[:, :, 0:64])
            bias_sb = singles.tile([128, 1], F32)
            wq2_sb = singles.tile([128, NKT, 128], BF16)
            ident = singles.tile([128, 128], F32R)

            def kv_a(c):
                xtc = xt_pool.tile([128, NKT, CH], BF16, name="xtc")
                nc.sync.dma_start(xtc[:, 0:4, :], xH[:, c, 0:4, :])
                nc.sync.dma_start(xtc[:, 4:8, :], xH[:, c, 4:8, :])
                if c == 0:
                    nc.sync.dma_start(bias_sb, biasd[:, :])
                    nc.sync.dma_start(wq2_sb, wq2[:, :, :])
                if c == 1:
                    nc.sync.dma_start(ident, identd[:, :])
                kve = kv_e_pool.tile([128, HC], F32, name="kve", tag="kvpe")
                kvo = kv_o_pool.tile([128, HC], F32, name="kvo")
                kvst[c] = [xtc, (kve, kvo), None]
                for kt in range(4):
                    nc.tensor.matmul(
                        kve,
                        wkv_e_sb[:, kt, :],
                        xtc[:, kt, 0:HC],
                        start=(kt == 0),
                        stop=False,
                    )
                    nc.tensor.matmul(
                        kvo,
                        wkv_o_sb[:, kt, :],
                        xtc[:, kt, HC:CH],
                        start=(kt == 0),
                        stop=False,
                    )

            def kv_b(c):
                xtc, (kve, kvo), _ = kvst[c]
                for kt in range(4, NKT):
                    nc.tensor.matmul(
                        kve,
                        wkv_e_sb[:, kt, :],
                        xtc[:, kt, 0:HC],
                        start=False,
                        stop=(kt == NKT - 1),
                    )
                    nc.tensor.matmul(
                        kvo,
                        wkv_o_sb[:, kt, :],
                        xtc[:, kt, HC:CH],
                        start=False,
                        stop=(kt == NKT - 1),
                    )
                # tokens 0:256: K on rows 0:64, V on 64:128 (even pack);
                # tokens 256:512: K on 64:128, V on 0:64 (odd pack).
                nc.vector.tensor_copy(
                    KT[0:64, (2 * c) * 128 : (2 * c + 2) * 128],
                    kve[0:64, :],
                )
                nc.vector.tensor_copy(
                    KT[64:128, (2 * c) * 128 : (2 * c + 2) * 128],
                    kvo[64:128, :],
                )
                vt = vt_pool.tile([128, CH], F32R, name="vt")
                nc.vector.tensor_copy(vt[64:128, 0:HC], kve[64:128, :])
                nc.vector.tensor_copy(vt[0:64, HC:CH], kvo[0:64, :])
                kvst[c][2] = vt

            def q_a(c):
                xtc = kvst[c][0]
                qp = oq_ps_pool.tile([128, CH], F32, name="qp", tag="oq")
                kvst[c].append(qp)
                for kt in range(4):
                    nc.tensor.matmul(
                        qp,
                        wq2_sb[:, kt, :],
                        xtc[:, kt, :],
                        start=(kt == 0),
                        stop=False,
                    )

            def q_b(c):
                xtc, _, _, qp = kvst[c]
                for kt in range(4, NKT):
                    nc.tensor.matmul(
                        qp,
                        wq2_sb[:, kt, :],
                        xtc[:, kt, :],
                        start=False,
                        stop=(kt == NKT - 1),
                    )
                nc.vector.tensor_scalar_add(
                    QT2[:, c * CH : (c + 1) * CH], qp, bias_sb[:, 0:1]
                )

            def kv_c(c):
                vt = kvst[c][2]
                for s4 in range(CH // 128):
                    vrows = slice(64, 128) if s4 < 2 else slice(0, 64)
                    t2 = kv_e_pool.tile([128, 128], F32, name="t2", tag="kvpe")
                    nc.tensor.transpose(
                        t2[:, 0:64].bitcast(F32R),
                        vt[vrows, s4 * 128 : (s4 + 1) * 128],
                        ident[vrows, vrows],
                    )
                    nc.vector.tensor_copy(
                        Vaug[:, c * (CH // 128) + s4, :], t2[:, 0:64]
                    )
                kvc_done[0] = c + 1

            # ---- single fused phase ----
            kv_a(0); kv_b(0); q_a(0); q_b(0)

            parts = [
                ("kva", 1), ("kvb", 1), ("qa", 1), ("qb", 1), ("kvc", 0),
                ("kva", 2), ("kvb", 2), ("qa", 2), ("qb", 2), ("kvc", 1),
                ("kva", 3), ("kvb", 3), ("qa", 3), ("qb", 3), ("kvc", 2),
                ("kva", 4), ("kvb", 4), ("kvc", 3),
                ("kva", 5), ("kvb", 5), ("kvc", 4),
                ("kva", 6), ("kvb", 6), ("kvc", 5),
                ("kva", 7), ("kvb", 7), ("kvc", 6), ("kvc", 7),
            ]
            part_fn = {"kva": kv_a, "kvb": kv_b, "qa": q_a, "qb": q_b,
                       "kvc": kv_c}
            part_want = {"kva": 2, "kvb": 2, "qa": 2, "qb": 2, "kvc": 2}
            kvb_done = 0
            qb_done = [True, False, False, False]
            next_pair = [0, 0, 0, 0]

            def slots_avail(qs):
                if not qb_done[qs]:
                    return False
                p = next_pair[qs]
                return p < NPAIR and p // 2 <= kvb_done

            def take_slot(qs):
                p = next_pair[qs]
                next_pair[qs] += 1
                emit_st(qs, p)

            num_ok = [False]
            den_ok = [True]

            def drains(floor, dcap=2, ncap=2):
                cutoff = slot_seq[0] - 3
                if den_ok[0]:
                    k = 0
                    while k < dcap and emit_den(cutoff):
                        k += 1
                if num_ok[0]:
                    k = 0
                    while k < ncap and (n_half[0] - n_num[0]) > floor and \
                            emit_num(cutoff):
                        k += 1

            for kind, c in parts:
                want = part_want[kind]
                for qs in range(NQS):
                    if want and slots_avail(qs):
                        take_slot(qs)
                        want -= 1
                part_fn[kind](c)
                if kind == "kvb":
                    kvb_done = c
                if kind == "qb":
                    qb_done[c] = True
                    if c == 3:
                        num_ok[0] = True
                drains(14)

            # ---- post-parts: remaining slots + decaying drain floor ----
            remaining = [(qs, p) for qs in range(NQS)
                         for p in range(next_pair[qs], NPAIR)]
            n_rem = len(remaining)
            num_ok[0] = True
            backlog0 = n_half[0] - n_num[0]
            for j, (qs, p) in enumerate(remaining):
                emit_st(qs, p)
                floor = max(2, (backlog0 * (n_rem - 1 - j)) // n_rem)
                drains(floor, dcap=2, ncap=3)
            inf = 1 << 30
            while emit_num(inf):
                pass
            while emit_den(inf):
                pass
            assert n_num[0] == NQS * NK2, n_num[0]
            assert n_den[0] == NQS * NK2, n_den[0]
    return nc


_NC_CACHE = None


def _get_nc():
    global _NC_CACHE
    if _NC_CACHE is None:
        nc = build_nc()
        nc.finalize()
        _NC_CACHE = nc
    return _NC_CACHE


LAST_RESULT = None
RUN_KWARGS = {}


def kernel(x, Wq, bq, Wk, bk, Wv, bv):
    global LAST_RESULT
    x = np.asarray(x, dtype=np.float32)
    Wq = np.asarray(Wq, dtype=np.float32)
    Wk = np.asarray(Wk, dtype=np.float32)
    Wv = np.asarray(Wv, dtype=np.float32)
    bq_a = np.asarray(bq, dtype=np.float32)
    bv_a = np.asarray(bv, dtype=np.float32)

    bf = ml_dtypes.bfloat16

    # per 128-row contraction tile [128, kt, 128]: [Wk|Wv]
    def pack2(wa, wb):
        h = np.empty((128, NKT, 128), np.float32)
        h[:, :, :64] = wa.reshape(NKT, 128, 64).transpose(1, 0, 2)
        h[:, :, 64:] = wb.reshape(NKT, 128, 64).transpose(1, 0, 2)
        return np.ascontiguousarray(h.reshape(128, NKT * 128)).astype(bf)

    wkv_e_host = pack2(Wk, Wv).reshape(128, NKT, 128)
    wq2_host = pack2(Wq, Wq).reshape(128, NKT, 128)
    bias_host = np.zeros((128, 1), np.float32)
    bias_host[:, 0] = np.concatenate([bq_a, bq_a])
    ident_host = np.eye(128, dtype=np.float32)

    in_maps = []
    for c in range(NCORES):
        b, h = divmod(c, 2)
        xb = x[b]
        if h == 1:
            xb = np.concatenate([xb[TQ:], xb[:TQ]], axis=0)
        # xH[p, c, k, t] = x^T[k*128+p, c*512+t]
        xh = np.ascontiguousarray(
            xb.T.astype(bf).reshape(NKT, 128, NCH, CH).transpose(1, 2, 0, 3)
        ).reshape(128, NCH, NKT, CH)
        in_maps.append(
            {
                "xH": xh,
                "wkv_e": wkv_e_host,
                "wq2": wq2_host,
                "biasd": bias_host,
                "identd": ident_host,
            }
        )

    nc = _get_nc()
    res = run_bass_kernel_spmd(nc, in_maps, core_ids=list(range(NCORES)), **RUN_KWARGS)
    LAST_RESULT = res

    outp = np.empty((B, S, H), np.float32)
    for c in range(NCORES):
        b, h = divmod(c, 2)
        on = np.asarray(res.results[c]["outn"], np.float32)  # [qs, 128, 512]
        num = on[:, 0:64, :] + on[:, 64:128, :]     # [qs, h, q]
        den = res.results[c]["outd"].sum(axis=1, keepdims=True)
        outp[b, h * TQ : (h + 1) * TQ] = (
            (num / den).transpose(0, 2, 1).reshape(TQ, H) + bv_a
        )
    return outp


# revision 37
# speedup vs baseline: 1.0787x; 1.0188x over previous
"""Single-head attention kernel for Trainium2, 8 NeuronCores.

Problem: x[4, 4096, 1024] f32; Wq/Wk/Wv [1024, 64]; bq/bk/bv [64].
  Q/K/V = x @ W + b ; out = softmax(Q K^T / 8) @ V  -> [4, 4096, 64]

Sharding: 8 shards = (batch b, query-half h). Each core computes K/V for
all 4096 tokens of its batch and attention for its 2048 queries.

v8 design (engine balance: ACT exp ~72us busy, PE ~75us busy):
  - Bias algebra: K-bias dropped (softmax invariant to per-query consts),
    V-bias added on the host (attn rows sum to 1). Only bq on device.
  - ONE uniform phase, no PSUM pool transition. Banks: score tiles
    2x[128,1024] (4) + kvp (1) + t2 (1) + oq (1) + den (1) = 8.
  - Split-pack projections: each 512-token chunk packs tokens 0:256
    with [Wk|Wv] (K on rows 0:64) and 256:512 with [Wv|Wk] (K on rows
    64:128), so a row-tiled score PAIR forms within ONE chunk - the
    first exp fires ~7us earlier (chunk 0 alone, not chunks 0+1).
    [Wv|Wk] is derived on-device from [Wk|Wv] by a DVE column swap.
  - Scores: row-tiled pairs -> S^T [128 keys, 1024 q] PSUM tiles; ACT
    exp (scale 1/8) -> bf16 P tiles; 64 uniform N=1024 slots.
  - AV numerator: col-tiled M=64 pairs (even arrivals -> psum[0:64],
    odd -> psum[64:128]; host adds). Denominator: 4x col-tiled M=1
    quads (ones lhsT) -> partitions 0/32/64/96 of the den bank; host
    sums. Drains only touch halves >=2 ACT slots old so pair/quad
    members are ready together and pop back-to-back (2x/4x).
  - Q projections run right after their own chunk, freeing the oq bank
    early so numerator drains span the whole kernel.
  - Output per qs: numerator [128, 512] bf16 (one DMA) + denominator
    partials (4 row DMAs); host reduces, divides, transposes, adds bv.
"""

from contextlib import ExitStack

import ml_dtypes
import numpy as np

import concourse.bass as bass
import concourse.mybir as mybir
from concourse import bacc
import concourse.tile as tile
from concourse.bass_utils import run_bass_kernel_spmd

B = 4
S = 4096
D = 1024
H = 64
NCORES = 8
TQ = S // 2      # queries per core
CH = 512         # token chunk for projections
HC = CH // 2     # split-pack half chunk
QS = 512         # query slice for attention
NKT = D // 128   # 8 contraction tiles for projections
NCH = S // CH    # 8 token chunks
NK2 = S // 128   # 32 key tiles for attention
NQS = TQ // QS   # 4 query slices
NPAIR = NK2 // 2  # 16 row-tiled score pairs per query slice
SCALE = 1.0 / 8.0  # 1/sqrt(64)

F32 = mybir.dt.float32
F32R = mybir.dt.float32r
BF16 = mybir.dt.bfloat16


def k2_of_slot(half, p):
    """Global key-tile index for pair p's lo/hi slot.

    Pair p lives in chunk p//2: lo = tokens (p%2)*128, hi = 256+(p%2)*128.
    """
    return 4 * (p // 2) + (2 if half else 0) + p % 2


def build_nc():
    nc = bacc.Bacc(None, target_bir_lowering=False)
    xH = nc.dram_tensor("xH", [128, NCH, NKT, CH], BF16, kind="ExternalInput")
    wkv_e = nc.dram_tensor("wkv_e", [128, NKT, 128], BF16, kind="ExternalInput")
    wq2 = nc.dram_tensor("wq2", [128, NKT, 128], BF16, kind="ExternalInput")
    biasd = nc.dram_tensor("biasd", [128, 1], F32, kind="ExternalInput")
    identd = nc.dram_tensor("identd", [128, 128], F32R, kind="ExternalInput")
    outn = nc.dram_tensor("outn", [NQS, 128, QS], BF16, kind="ExternalOutput")
    outd = nc.dram_tensor("outd", [NQS, 4, QS], F32, kind="ExternalOutput")

    with ExitStack() as ctx:
        tc = ctx.enter_context(tile.TileContext(nc))
        singles = ctx.enter_context(tc.tile_pool(name="singles", bufs=1))
        persist = ctx.enter_context(tc.tile_pool(name="persist", bufs=1))

        KT = persist.tile([128, NPAIR * 128], BF16)
        QT2 = persist.tile([128, TQ], BF16)     # Q^T duplicated on both halves
        Vaug = persist.tile([128, NK2, 64], BF16)  # V natural layout

        with (
            tc.tile_pool(name="xt", bufs=3) as xt_pool,
            tc.tile_pool(name="vt", bufs=8) as vt_pool,
            tc.tile_pool(name="p", bufs=34) as p_pool,
            tc.tile_pool(name="osb", bufs=2) as osb_pool,
            tc.tile_pool(name="osbd", bufs=2) as osbd_pool,
            tc.tile_pool(name="stps", bufs=2, space="PSUM") as st_pool,
            tc.tile_pool(name="kvpe", bufs=1, space="PSUM") as kv_e_pool,
            tc.tile_pool(name="kvpo", bufs=1, space="PSUM") as kv_o_pool,
            tc.tile_pool(name="oqps", bufs=1, space="PSUM") as oq_ps_pool,
            tc.tile_pool(name="dnps", bufs=1, space="PSUM") as dn_ps_pool,
        ):
            kvst = {}  # c -> [xtc, kvp, vt, (qp)] chunk state

            # qs -> [(k2, view512, slot_id)]
            arrivals = {q: [] for q in range(NQS)}
            slot_seq = [0]
            num_cur = [0 for _ in range(NQS)]
            den_cur = [0 for _ in range(NQS)]
            num_ptr = [0]
            den_ptr = [0]
            num_st = {}
            den_st = {}
            n_half = [0]
            n_num = [0]
            n_den = [0]
            kvc_done = [0]  # chunks whose V tiles are in Vaug

            def emit_st(qs, p):
                st = st_pool.tile([128, 2 * QS], F32, name="st")
                for i, half in enumerate((0, 1)):
                    rows = slice(64, 128) if half else slice(0, 64)
                    nc.tensor.matmul(
                        st[:, i * QS : (i + 1) * QS],
                        KT[rows, p * 128 : (p + 1) * 128],
                        QT2[rows, qs * QS : (qs + 1) * QS],
                        start=True,
                        stop=True,
                    )
                p_tile = p_pool.tile([128, 2 * QS], BF16, name="pt")
                nc.scalar.activation(
                    p_tile, st, mybir.ActivationFunctionType.Exp, scale=SCALE
                )
                for i, half in enumerate((0, 1)):
                    arrivals[qs].append(
                        (k2_of_slot(half, p),
                         p_tile[:, i * QS : (i + 1) * QS],
                         slot_seq[0])
                    )
                slot_seq[0] += 1
                n_half[0] += 2

            def n_ready(qs, cur, cutoff, vaug=False):
                n = 0
                for e in arrivals[qs][cur:]:
                    if e[2] > cutoff:
                        break
                    if vaug and e[0] // 4 >= kvc_done[0]:
                        break
                    n += 1
                return n

            def emit_num(cutoff):
                qs = num_ptr[0]
                if qs >= NQS:
                    return False
                avail = n_ready(qs, num_cur[qs], cutoff, vaug=True)
                last = num_cur[qs] + avail >= NK2
                if avail < 2 and not (avail == 1 and last):
                    return False
                if qs not in num_st:
                    num_st[qs] = [
                        oq_ps_pool.tile([128, QS], F32, name="op", tag="oq"),
                        0,
                        0,
                    ]
                st = num_st[qs]
                take = min(2, avail)
                for j in range(take):
                    k2, view, _ = arrivals[qs][num_cur[qs]]
                    num_cur[qs] += 1
                    grp = j if take == 2 else (0 if st[1] < NK2 // 2 else 1)
                    n = st[1 + grp]
                    rows = slice(0, 64) if grp == 0 else slice(64, 128)
                    nc.tensor.matmul(
                        st[0][rows, :],
                        Vaug[:, k2, :],
                        view,
                        start=(n == 0),
                        stop=(n == NK2 // 2 - 1),
                    )
                    st[1 + grp] += 1
                    n_num[0] += 1
                if num_cur[qs] == NK2:
                    osb = osb_pool.tile([128, QS], BF16, name="osb")
                    nc.vector.tensor_copy(osb, st[0])
                    nc.sync.dma_start(outn[qs, :, :], osb)
                    del num_st[qs]
                    num_ptr[0] += 1
                return True

            def emit_den(cutoff):
                qs = den_ptr[0]
                if qs >= NQS:
                    return False
                avail = n_ready(qs, den_cur[qs], cutoff)
                if avail < 4:
                    return False
                if qs not in den_st:
                    den_st[qs] = [
                        dn_ps_pool.tile([128, QS], F32, name="dn", tag="dn"),
                        [0, 0, 0, 0],
                    ]
                st = den_st[qs]
                for j in range(4):
                    _, view, _ = arrivals[qs][den_cur[qs]]
                    den_cur[qs] += 1
                    n = st[1][j]
                    nc.tensor.matmul(
                        st[0][32 * j : 32 * j + 1, :],
                        ones1[:, 0:1],
                        view,
                        start=(n == 0),
                        stop=(n == NK2 // 4 - 1),
                        tile_position=(0, 32 * j),
                    )
                    st[1][j] += 1
                    n_den[0] += 1
                if den_cur[qs] == NK2:
                    osbd = osbd_pool.tile([97, QS], F32, name="osbd")
                    nc.vector.tensor_copy(osbd, st[0][0:97, :])
                    for i in range(4):
                        nc.sync.dma_start(
                            outd[qs, i : i + 1, :],
                            osbd[32 * i : 32 * i + 1, :],
                        )
                    del den_st[qs]
                    den_ptr[0] += 1
                return True

            # Warmup: preload the exp ACT table and run matmuls with no
            # readers so the PE HAM un-throttles during the DMA head.
            wrm = singles.tile([128, QS], BF16)
            nc.vector.memset(wrm, 0.0)
            wrm2 = singles.tile([128, 32], BF16)
            nc.scalar.activation(wrm2, wrm[:, 0:32],
                                 mybir.ActivationFunctionType.Exp,
                                 scale=SCALE)
            for _ in range(18):
                wps = st_pool.tile([128, 2 * QS], F32, name="st")
                nc.tensor.matmul(wps[:, 0:QS], wrm[:, 0:128], wrm,
                                 start=True, stop=True)
            ones1 = singles.tile([128, 1], BF16)
            nc.vector.memset(ones1, 1.0)

            # DMA issue order = queue-FIFO transfer order: wkv_e, c0(A,B),
            # wq2+bias, c1(A,B), ident, c2, ...  [Wv|Wk] is DVE-derived.
            wkv_e_sb = singles.tile([128, NKT, 128], BF16)
            nc.sync.dma_start(wkv_e_sb, wkv_e[:, :, :])
            wkv_o_sb = singles.tile([128, NKT, 128], BF16)
            nc.vector.tensor_copy(wkv_o_sb[:, :, 0:64], wkv_e_sb[:, :, 64:128])
            nc.vector.tensor_copy(wkv_o_sb[:, :, 64:128], wkv_e_sb[:, :, 0:64])
            bias_sb = singles.tile([128, 1], F32)
            wq2_sb = singles.tile([128, NKT, 128], BF16)
            ident = singles.tile([128, 128], F32R)

            def kv_a(c):
                xtc = xt_pool.tile([128, NKT, CH], BF16, name="xtc")
                nc.sync.dma_start(xtc[:, 0:4, :], xH[:, c, 0:4, :])
                nc.sync.dma_start(xtc[:, 4:8, :], xH[:, c, 4:8, :])
                if c == 0:
                    nc.sync.dma_start(bias_sb, biasd[:, :])
                    nc.sync.dma_start(wq2_sb, wq2[:, :, :])
                if c == 1:
                    nc.sync.dma_start(ident, identd[:, :])
                kve = kv_e_pool.tile([128, HC], F32, name="kve", tag="kvpe")
                kvo = kv_o_pool.tile([128, HC], F32, name="kvo")
                kvst[c] = [xtc, (kve, kvo), None]
                for kt in range(4):
                    nc.tensor.matmul(
                        kve,
                        wkv_e_sb[:, kt, :],
                        xtc[:, kt, 0:HC],
                        start=(kt == 0),
                        stop=False,
                    )
                    nc.tensor.matmul(
                        kvo,
                        wkv_o_sb[:, kt, :],
                        xtc[:, kt, HC:CH],
                        start=(kt == 0),
                        stop=False,
                    )

            def kv_b(c):
                xtc, (kve, kvo), _ = kvst[c]
                for kt in range(4, NKT):
                    nc.tensor.matmul(
                        kve,
                        wkv_e_sb[:, kt, :],
                        xtc[:, kt, 0:HC],
                        start=False,
                        stop=(kt == NKT - 1),
                    )
                    nc.tensor.matmul(
                        kvo,
                        wkv_o_sb[:, kt, :],
                        xtc[:, kt, HC:CH],
                        start=False,
                        stop=(kt == NKT - 1),
                    )
                # tokens 0:256: K on rows 0:64, V on 64:128 (even pack);
                # tokens 256:512: K on 64:128, V on 0:64 (odd pack).
                nc.vector.tensor_copy(
                    KT[0:64, (2 * c) * 128 : (2 * c + 2) * 128],
                    kve[0:64, :],
                )
                nc.vector.tensor_copy(
                    KT[64:128, (2 * c) * 128 : (2 * c + 2) * 128],
                    kvo[64:128, :],
                )
                vt = vt_pool.tile([128, CH], F32R, name="vt")
                nc.vector.tensor_copy(vt[64:128, 0:HC], kve[64:128, :])
                nc.vector.tensor_copy(vt[0:64, HC:CH], kvo[0:64, :])
                kvst[c][2] = vt

            def q_a(c):
                xtc = kvst[c][0]
                qp = oq_ps_pool.tile([128, CH], F32, name="qp", tag="oq")
                kvst[c].append(qp)
                for kt in range(4):
                    nc.tensor.matmul(
                        qp,
                        wq2_sb[:, kt, :],
                        xtc[:, kt, :],
                        start=(kt == 0),
                        stop=False,
                    )

            def q_b(c):
                xtc, _, _, qp = kvst[c]
                for kt in range(4, NKT):
                    nc.tensor.matmul(
                        qp,
                        wq2_sb[:, kt, :],
                        xtc[:, kt, :],
                        start=False,
                        stop=(kt == NKT - 1),
                    )
                nc.vector.tensor_scalar_add(
                    QT2[:, c * CH : (c + 1) * CH], qp, bias_sb[:, 0:1]
                )

            def kv_c(c):
                vt = kvst[c][2]
                for s4 in range(CH // 128):
                    vrows = slice(64, 128) if s4 < 2 else slice(0, 64)
                    t2 = kv_e_pool.tile([128, 128], F32, name="t2", tag="kvpe")
                    nc.tensor.transpose(
                        t2[:, 0:64].bitcast(F32R),
                        vt[vrows, s4 * 128 : (s4 + 1) * 128],
                        ident[vrows, vrows],
                    )
                    nc.vector.tensor_copy(
                        Vaug[:, c * (CH // 128) + s4, :], t2[:, 0:64]
                    )
                kvc_done[0] = c + 1

            # ---- single fused phase ----
            kv_a(0); kv_b(0); q_a(0); q_b(0)

            parts = [
                ("kva", 1), ("kvb", 1), ("qa", 1), ("qb", 1), ("kvc", 0),
                ("kva", 2), ("kvb", 2), ("qa", 2), ("qb", 2), ("kvc", 1),
                ("kva", 3), ("kvb", 3), ("qa", 3), ("qb", 3), ("kvc", 2),
                ("kva", 4), ("kvb", 4), ("kvc", 3),
                ("kva", 5), ("kvb", 5), ("kvc", 4),
                ("kva", 6), ("kvb", 6), ("kvc", 5),
                ("kva", 7), ("kvb", 7), ("kvc", 6), ("kvc", 7),
            ]
            part_fn = {"kva": kv_a, "kvb": kv_b, "qa": q_a, "qb": q_b,
                       "kvc": kv_c}
            part_want = {"kva": 2, "kvb": 2, "qa": 2, "qb": 2, "kvc": 2}
            kvb_done = 0
            qb_done = [True, False, False, False]
            next_pair = [0, 0, 0, 0]

            def slots_avail(qs):
                if not qb_done[qs]:
                    return False
                p = next_pair[qs]
                return p < NPAIR and p // 2 <= kvb_done

            def take_slot(qs):
                p = next_pair[qs]
                next_pair[qs] += 1
                emit_st(qs, p)

            num_ok = [False]
            den_ok = [True]

            def drains(floor, dcap=2, ncap=2):
                cutoff = slot_seq[0] - 3
                if den_ok[0]:
                    k = 0
                    while k < dcap and emit_den(cutoff):
                        k += 1
                if num_ok[0]:
                    k = 0
                    while k < ncap and (n_half[0] - n_num[0]) > floor and \
                            emit_num(cutoff):
                        k += 1

            for kind, c in parts:
                want = part_want[kind]
                for qs in range(NQS):
                    if want and slots_avail(qs):
                        take_slot(qs)
                        want -= 1
                part_fn[kind](c)
                if kind == "kvb":
                    kvb_done = c
                if kind == "qb":
                    qb_done[c] = True
                    if c == 3:
                        num_ok[0] = True
                drains(14)

            # ---- post-parts: remaining slots + decaying drain floor ----
            remaining = [(qs, p) for qs in range(NQS)
                         for p in range(next_pair[qs], NPAIR)]
            n_rem = len(remaining)
            num_ok[0] = True
            backlog0 = n_half[0] - n_num[0]
            for j, (qs, p) in enumerate(remaining):
                emit_st(qs, p)
                floor = max(2, (backlog0 * (n_rem - 1 - j)) // n_rem)
                drains(floor, dcap=1, ncap=2)
            inf = 1 << 30
            while emit_num(inf):
                pass
            while emit_den(inf):
                pass
            assert n_num[0] == NQS * NK2, n_num[0]
            assert n_den[0] == NQS * NK2, n_den[0]
    return nc


_NC_CACHE = None


def _get_nc():
    global _NC_CACHE
    if _NC_CACHE is None:
        nc = build_nc()
        nc.finalize()
        _NC_CACHE = nc
    return _NC_CACHE


LAST_RESULT = None
RUN_KWARGS = {}


def kernel(x, Wq, bq, Wk, bk, Wv, bv):
    global LAST_RESULT
    x = np.asarray(x, dtype=np.float32)
    Wq = np.asarray(Wq, dtype=np.float32)
    Wk = np.asarray(Wk, dtype=np.float32)
    Wv = np.asarray(Wv, dtype=np.float32)
    bq_a = np.asarray(bq, dtype=np.float32)
    bv_a = np.asarray(bv, dtype=np.float32)

    bf = ml_dtypes.bfloat16

    # per 128-row contraction tile [128, kt, 128]: [Wk|Wv]
    def pack2(wa, wb):
        h = np.empty((128, NKT, 128), np.float32)
        h[:, :, :64] = wa.reshape(NKT, 128, 64).transpose(1, 0, 2)
        h[:, :, 64:] = wb.reshape(NKT, 128, 64).transpose(1, 0, 2)
        return np.ascontiguousarray(h.reshape(128, NKT * 128)).astype(bf)

    wkv_e_host = pack2(Wk, Wv).reshape(128, NKT, 128)
    wq2_host = pack2(Wq, Wq).reshape(128, NKT, 128)
    bias_host = np.zeros((128, 1), np.float32)
    bias_host[:, 0] = np.concatenate([bq_a, bq_a])
    ident_host = np.eye(128, dtype=np.float32)

    in_maps = []
    for c in range(NCORES):
        b, h = divmod(c, 2)
        xb = x[b]
        if h == 1:
            xb = np.concatenate([xb[TQ:], xb[:TQ]], axis=0)
        # xH[p, c, k, t] = x^T[k*128+p, c*512+t]
        xh = np.ascontiguousarray(
            xb.T.astype(bf).reshape(NKT, 128, NCH, CH).transpose(1, 2, 0, 3)
        ).reshape(128, NCH, NKT, CH)
        in_maps.append(
            {
                "xH": xh,
                "wkv_e": wkv_e_host,
                "wq2": wq2_host,
                "biasd": bias_host,
                "identd": ident_host,
            }
        )

    nc = _get_nc()
    res = run_bass_kernel_spmd(nc, in_maps, core_ids=list(range(NCORES)), **RUN_KWARGS)
    LAST_RESULT = res

    outp = np.empty((B, S, H), np.float32)
    for c in range(NCORES):
        b, h = divmod(c, 2)
        on = np.asarray(res.results[c]["outn"], np.float32)  # [qs, 128, 512]
        num = on[:, 0:64, :] + on[:, 64:128, :]     # [qs, h, q]
        den = res.results[c]["outd"].sum(axis=1, keepdims=True)
        outp[b, h * TQ : (h + 1) * TQ] = (
            (num / den).transpose(0, 2, 1).reshape(TQ, H) + bv_a
        )
    return outp


# revision 38
# speedup vs baseline: 1.0818x; 1.0029x over previous
"""Single-head attention kernel for Trainium2, 8 NeuronCores.

Problem: x[4, 4096, 1024] f32; Wq/Wk/Wv [1024, 64]; bq/bk/bv [64].
  Q/K/V = x @ W + b ; out = softmax(Q K^T / 8) @ V  -> [4, 4096, 64]

Sharding: 8 shards = (batch b, query-half h). Each core computes K/V for
all 4096 tokens of its batch and attention for its 2048 queries.

v8 design (engine balance: ACT exp ~72us busy, PE ~75us busy):
  - Bias algebra: K-bias dropped (softmax invariant to per-query consts),
    V-bias added on the host (attn rows sum to 1). Only bq on device.
  - ONE uniform phase, no PSUM pool transition. Banks: score tiles
    2x[128,1024] (4) + kvp (1) + t2 (1) + oq (1) + den (1) = 8.
  - Split-pack projections: each 512-token chunk packs tokens 0:256
    with [Wk|Wv] (K on rows 0:64) and 256:512 with [Wv|Wk] (K on rows
    64:128), so a row-tiled score PAIR forms within ONE chunk - the
    first exp fires ~7us earlier (chunk 0 alone, not chunks 0+1).
    [Wv|Wk] is derived on-device from [Wk|Wv] by a DVE column swap.
  - Scores: row-tiled pairs -> S^T [128 keys, 1024 q] PSUM tiles; ACT
    exp (scale 1/8) -> bf16 P tiles; 64 uniform N=1024 slots.
  - AV numerator: col-tiled M=64 pairs (even arrivals -> psum[0:64],
    odd -> psum[64:128]; host adds). Denominator: 4x col-tiled M=1
    quads (ones lhsT) -> partitions 0/32/64/96 of the den bank; host
    sums. Drains only touch halves >=2 ACT slots old so pair/quad
    members are ready together and pop back-to-back (2x/4x).
  - Q projections run right after their own chunk, freeing the oq bank
    early so numerator drains span the whole kernel.
  - Output per qs: numerator [128, 512] bf16 (one DMA) + denominator
    partials (4 row DMAs); host reduces, divides, transposes, adds bv.
"""

from contextlib import ExitStack

import ml_dtypes
import numpy as np

import concourse.bass as bass
import concourse.mybir as mybir
from concourse import bacc
import concourse.tile as tile
from concourse.bass_utils import run_bass_kernel_spmd

B = 4
S = 4096
D = 1024
H = 64
NCORES = 8
TQ = S // 2      # queries per core
CH = 512         # token chunk for projections
HC = CH // 2     # split-pack half chunk
QS = 512         # query slice for attention
NKT = D // 128   # 8 contraction tiles for projections
NCH = S // CH    # 8 token chunks
NK2 = S // 128   # 32 key tiles for attention
NQS = TQ // QS   # 4 query slices
NPAIR = NK2 // 2  # 16 row-tiled score pairs per query slice
SCALE = 1.0 / 8.0  # 1/sqrt(64)

F32 = mybir.dt.float32
F32R = mybir.dt.float32r
BF16 = mybir.dt.bfloat16


def k2_of_slot(half, p):
    """Global key-tile index for pair p's lo/hi slot.

    Pair p lives in chunk p//2: lo = tokens (p%2)*128, hi = 256+(p%2)*128.
    """
    return 4 * (p // 2) + (2 if half else 0) + p % 2


def build_nc():
    nc = bacc.Bacc(None, target_bir_lowering=False)
    xH = nc.dram_tensor("xH", [128, NCH, NKT, CH], BF16, kind="ExternalInput")
    wkv_e = nc.dram_tensor("wkv_e", [128, NKT, 128], BF16, kind="ExternalInput")
    wq2 = nc.dram_tensor("wq2", [128, NKT, 128], BF16, kind="ExternalInput")
    biasd = nc.dram_tensor("biasd", [128, 1], F32, kind="ExternalInput")
    identd = nc.dram_tensor("identd", [128, 128], F32R, kind="ExternalInput")
    outn = nc.dram_tensor("outn", [NQS, 128, QS], BF16, kind="ExternalOutput")
    outd = nc.dram_tensor("outd", [NQS, 4, QS], F32, kind="ExternalOutput")

    with ExitStack() as ctx:
        tc = ctx.enter_context(tile.TileContext(nc))
        singles = ctx.enter_context(tc.tile_pool(name="singles", bufs=1))
        persist = ctx.enter_context(tc.tile_pool(name="persist", bufs=1))

        KT = persist.tile([128, NPAIR * 128], BF16)
        QT2 = persist.tile([128, TQ], BF16)     # Q^T duplicated on both halves
        Vaug = persist.tile([128, NK2, 64], BF16)  # V natural layout

        with (
            tc.tile_pool(name="xt", bufs=3) as xt_pool,
            tc.tile_pool(name="vt", bufs=8) as vt_pool,
            tc.tile_pool(name="p", bufs=34) as p_pool,
            tc.tile_pool(name="osb", bufs=2) as osb_pool,
            tc.tile_pool(name="osbd", bufs=2) as osbd_pool,
            tc.tile_pool(name="stps", bufs=2, space="PSUM") as st_pool,
            tc.tile_pool(name="kvpe", bufs=1, space="PSUM") as kv_e_pool,
            tc.tile_pool(name="kvpo", bufs=1, space="PSUM") as kv_o_pool,
            tc.tile_pool(name="oqps", bufs=1, space="PSUM") as oq_ps_pool,
            tc.tile_pool(name="dnps", bufs=1, space="PSUM") as dn_ps_pool,
        ):
            kvst = {}  # c -> [xtc, kvp, vt, (qp)] chunk state

            # qs -> [(k2, view512, slot_id)]
            arrivals = {q: [] for q in range(NQS)}
            slot_seq = [0]
            num_cur = [0 for _ in range(NQS)]
            den_cur = [0 for _ in range(NQS)]
            num_ptr = [0]
            den_ptr = [0]
            num_st = {}
            den_st = {}
            n_half = [0]
            n_num = [0]
            n_den = [0]
            kvc_done = [0]  # chunks whose V tiles are in Vaug

            def emit_st(qs, p):
                st = st_pool.tile([128, 2 * QS], F32, name="st")
                for i, half in enumerate((0, 1)):
                    rows = slice(64, 128) if half else slice(0, 64)
                    nc.tensor.matmul(
                        st[:, i * QS : (i + 1) * QS],
                        KT[rows, p * 128 : (p + 1) * 128],
                        QT2[rows, qs * QS : (qs + 1) * QS],
                        start=True,
                        stop=True,
                    )
                p_tile = p_pool.tile([128, 2 * QS], BF16, name="pt")
                nc.scalar.activation(
                    p_tile, st, mybir.ActivationFunctionType.Exp, scale=SCALE
                )
                for i, half in enumerate((0, 1)):
                    arrivals[qs].append(
                        (k2_of_slot(half, p),
                         p_tile[:, i * QS : (i + 1) * QS],
                         slot_seq[0])
                    )
                slot_seq[0] += 1
                n_half[0] += 2

            def n_ready(qs, cur, cutoff, vaug=False):
                n = 0
                for e in arrivals[qs][cur:]:
                    if e[2] > cutoff:
                        break
                    if vaug and e[0] // 4 >= kvc_done[0]:
                        break
                    n += 1
                return n

            def emit_num(cutoff):
                qs = num_ptr[0]
                if qs >= NQS:
                    return False
                avail = n_ready(qs, num_cur[qs], cutoff, vaug=True)
                last = num_cur[qs] + avail >= NK2
                if avail < 2 and not (avail == 1 and last):
                    return False
                if qs not in num_st:
                    num_st[qs] = [
                        oq_ps_pool.tile([128, QS], F32, name="op", tag="oq"),
                        0,
                        0,
                    ]
                st = num_st[qs]
                take = min(2, avail)
                for j in range(take):
                    k2, view, _ = arrivals[qs][num_cur[qs]]
                    num_cur[qs] += 1
                    grp = j if take == 2 else (0 if st[1] < NK2 // 2 else 1)
                    n = st[1 + grp]
                    rows = slice(0, 64) if grp == 0 else slice(64, 128)
                    nc.tensor.matmul(
                        st[0][rows, :],
                        Vaug[:, k2, :],
                        view,
                        start=(n == 0),
                        stop=(n == NK2 // 2 - 1),
                    )
                    st[1 + grp] += 1
                    n_num[0] += 1
                if num_cur[qs] == NK2:
                    osb = osb_pool.tile([128, QS], BF16, name="osb")
                    nc.vector.tensor_copy(osb, st[0])
                    nc.sync.dma_start(outn[qs, :, :], osb)
                    del num_st[qs]
                    num_ptr[0] += 1
                return True

            def emit_den(cutoff):
                qs = den_ptr[0]
                if qs >= NQS:
                    return False
                avail = n_ready(qs, den_cur[qs], cutoff)
                if avail < 4:
                    return False
                if qs not in den_st:
                    den_st[qs] = [
                        dn_ps_pool.tile([128, QS], F32, name="dn", tag="dn"),
                        [0, 0, 0, 0],
                    ]
                st = den_st[qs]
                for j in range(4):
                    _, view, _ = arrivals[qs][den_cur[qs]]
                    den_cur[qs] += 1
                    n = st[1][j]
                    nc.tensor.matmul(
                        st[0][32 * j : 32 * j + 1, :],
                        ones1[:, 0:1],
                        view,
                        start=(n == 0),
                        stop=(n == NK2 // 4 - 1),
                        tile_position=(0, 32 * j),
                    )
                    st[1][j] += 1
                    n_den[0] += 1
                if den_cur[qs] == NK2:
                    osbd = osbd_pool.tile([97, QS], F32, name="osbd")
                    nc.vector.tensor_copy(osbd, st[0][0:97, :])
                    for i in range(4):
                        nc.sync.dma_start(
                            outd[qs, i : i + 1, :],
                            osbd[32 * i : 32 * i + 1, :],
                        )
                    del den_st[qs]
                    den_ptr[0] += 1
                return True

            # Warmup: preload the exp ACT table and run matmuls with no
            # readers so the PE HAM un-throttles during the DMA head.
            wrm = singles.tile([128, QS], BF16)
            nc.vector.memset(wrm, 0.0)
            wrm2 = singles.tile([128, 32], BF16)
            nc.scalar.activation(wrm2, wrm[:, 0:32],
                                 mybir.ActivationFunctionType.Exp,
                                 scale=SCALE)
            for _ in range(18):
                wps = st_pool.tile([128, 2 * QS], F32, name="st")
                nc.tensor.matmul(wps[:, 0:QS], wrm[:, 0:128], wrm,
                                 start=True, stop=True)
            ones1 = singles.tile([128, 1], BF16)
            nc.vector.memset(ones1, 1.0)

            # DMA issue order = queue-FIFO transfer order: wkv_e, c0(A,B),
            # wq2+bias, c1(A,B), ident, c2, ...  [Wv|Wk] is DVE-derived.
            wkv_e_sb = singles.tile([128, NKT, 128], BF16)
            nc.sync.dma_start(wkv_e_sb, wkv_e[:, :, :])
            wkv_o_sb = singles.tile([128, NKT, 128], BF16)
            nc.vector.tensor_copy(wkv_o_sb[:, :, 0:64], wkv_e_sb[:, :, 64:128])
            nc.vector.tensor_copy(wkv_o_sb[:, :, 64:128], wkv_e_sb[:, :, 0:64])
            bias_sb = singles.tile([128, 1], F32)
            wq2_sb = singles.tile([128, NKT, 128], BF16)
            ident = singles.tile([128, 128], F32R)

            def kv_a(c):
                xtc = xt_pool.tile([128, NKT, CH], BF16, name="xtc")
                nc.sync.dma_start(xtc[:, 0:4, :], xH[:, c, 0:4, :])
                nc.sync.dma_start(xtc[:, 4:8, :], xH[:, c, 4:8, :])
                if c == 0:
                    nc.sync.dma_start(bias_sb, biasd[:, :])
                    nc.sync.dma_start(wq2_sb, wq2[:, :, :])
                if c == 1:
                    nc.sync.dma_start(ident, identd[:, :])
                kve = kv_e_pool.tile([128, HC], F32, name="kve", tag="kvpe")
                kvo = kv_o_pool.tile([128, HC], F32, name="kvo")
                kvst[c] = [xtc, (kve, kvo), None]
                for kt in range(4):
                    nc.tensor.matmul(
                        kve,
                        wkv_e_sb[:, kt, :],
                        xtc[:, kt, 0:HC],
                        start=(kt == 0),
                        stop=False,
                    )
                    nc.tensor.matmul(
                        kvo,
                        wkv_o_sb[:, kt, :],
                        xtc[:, kt, HC:CH],
                        start=(kt == 0),
                        stop=False,
                    )

            def kv_b(c):
                xtc, (kve, kvo), _ = kvst[c]
                for kt in range(4, NKT):
                    nc.tensor.matmul(
                        kve,
                        wkv_e_sb[:, kt, :],
                        xtc[:, kt, 0:HC],
                        start=False,
                        stop=(kt == NKT - 1),
                    )
                    nc.tensor.matmul(
                        kvo,
                        wkv_o_sb[:, kt, :],
                        xtc[:, kt, HC:CH],
                        start=False,
                        stop=(kt == NKT - 1),
                    )
                # tokens 0:256: K on rows 0:64, V on 64:128 (even pack);
                # tokens 256:512: K on 64:128, V on 0:64 (odd pack).
                nc.vector.tensor_copy(
                    KT[0:64, (2 * c) * 128 : (2 * c + 2) * 128],
                    kve[0:64, :],
                )
                nc.vector.tensor_copy(
                    KT[64:128, (2 * c) * 128 : (2 * c + 2) * 128],
                    kvo[64:128, :],
                )
                vt = vt_pool.tile([128, CH], F32R, name="vt")
                nc.vector.tensor_copy(vt[64:128, 0:HC], kve[64:128, :])
                nc.vector.tensor_copy(vt[0:64, HC:CH], kvo[0:64, :])
                kvst[c][2] = vt

            def q_a(c):
                xtc = kvst[c][0]
                qp = oq_ps_pool.tile([128, CH], F32, name="qp", tag="oq")
                kvst[c].append(qp)
                for kt in range(4):
                    nc.tensor.matmul(
                        qp,
                        wq2_sb[:, kt, :],
                        xtc[:, kt, :],
                        start=(kt == 0),
                        stop=False,
                    )

            def q_b(c):
                xtc, _, _, qp = kvst[c]
                for kt in range(4, NKT):
                    nc.tensor.matmul(
                        qp,
                        wq2_sb[:, kt, :],
                        xtc[:, kt, :],
                        start=False,
                        stop=(kt == NKT - 1),
                    )
                nc.vector.tensor_scalar_add(
                    QT2[:, c * CH : (c + 1) * CH], qp, bias_sb[:, 0:1]
                )

            def kv_c(c):
                vt = kvst[c][2]
                for s4 in range(CH // 128):
                    vrows = slice(64, 128) if s4 < 2 else slice(0, 64)
                    t2 = kv_e_pool.tile([128, 128], F32, name="t2", tag="kvpe")
                    nc.tensor.transpose(
                        t2[:, 0:64].bitcast(F32R),
                        vt[vrows, s4 * 128 : (s4 + 1) * 128],
                        ident[vrows, vrows],
                    )
                    nc.vector.tensor_copy(
                        Vaug[:, c * (CH // 128) + s4, :], t2[:, 0:64]
                    )
                kvc_done[0] = c + 1

            # ---- single fused phase ----
            kv_a(0); kv_b(0); q_a(0); q_b(0)

            parts = [
                ("kva", 1), ("kvb", 1), ("qa", 1), ("qb", 1), ("kvc", 0),
                ("kva", 2), ("kvb", 2), ("qa", 2), ("qb", 2), ("kvc", 1),
                ("kva", 3), ("kvb", 3), ("qa", 3), ("qb", 3), ("kvc", 2),
                ("kva", 4), ("kvb", 4), ("kvc", 3),
                ("kva", 5), ("kvb", 5), ("kvc", 4),
                ("kva", 6), ("kvb", 6), ("kvc", 5),
                ("kva", 7), ("kvb", 7), ("kvc", 6), ("kvc", 7),
            ]
            part_fn = {"kva": kv_a, "kvb": kv_b, "qa": q_a, "qb": q_b,
                       "kvc": kv_c}
            part_want = {"kva": 2, "kvb": 2, "qa": 2, "qb": 2, "kvc": 2}
            kvb_done = 0
            qb_done = [True, False, False, False]
            next_pair = [0, 0, 0, 0]

            def slots_avail(qs):
                if not qb_done[qs]:
                    return False
                p = next_pair[qs]
                return p < NPAIR and p // 2 <= kvb_done

            def take_slot(qs):
                p = next_pair[qs]
                next_pair[qs] += 1
                emit_st(qs, p)

            num_ok = [False]
            den_ok = [True]

            def drains(floor, dcap=2, ncap=2):
                cutoff = slot_seq[0] - 3
                if den_ok[0]:
                    k = 0
                    while k < dcap and emit_den(cutoff):
                        k += 1
                if num_ok[0]:
                    k = 0
                    while k < ncap and (n_half[0] - n_num[0]) > floor and \
                            emit_num(cutoff):
                        k += 1

            for kind, c in parts:
                want = part_want[kind]
                for qs in range(NQS):
                    if want and slots_avail(qs):
                        take_slot(qs)
                        want -= 1
                part_fn[kind](c)
                if kind == "kvb":
                    kvb_done = c
                if kind == "qb":
                    qb_done[c] = True
                    if c == 3:
                        num_ok[0] = True
                drains(14)

            # ---- post-parts: remaining slots + decaying drain floor ----
            remaining = [(qs, p) for qs in range(NQS)
                         for p in range(next_pair[qs], NPAIR)]
            n_rem = len(remaining)
            num_ok[0] = True
            backlog0 = n_half[0] - n_num[0]
            for j, (qs, p) in enumerate(remaining):
                emit_st(qs, p)
                floor = max(0, (backlog0 * (n_rem - 1 - j)) // n_rem)
                late = j >= (n_rem * 11) // 16
                drains(floor, dcap=2 if late else 1, ncap=3 if late else 2)
            inf = 1 << 30
            while emit_num(inf):
                pass
            while emit_den(inf):
                pass
            assert n_num[0] == NQS * NK2, n_num[0]
            assert n_den[0] == NQS * NK2, n_den[0]
    return nc


_NC_CACHE = None


def _get_nc():
    global _NC_CACHE
    if _NC_CACHE is None:
        nc = build_nc()
        nc.finalize()
        _NC_CACHE = nc
    return _NC_CACHE


LAST_RESULT = None
RUN_KWARGS = {}


def kernel(x, Wq, bq, Wk, bk, Wv, bv):
    global LAST_RESULT
    x = np.asarray(x, dtype=np.float32)
    Wq = np.asarray(Wq, dtype=np.float32)
    Wk = np.asarray(Wk, dtype=np.float32)
    Wv = np.asarray(Wv, dtype=np.float32)
    bq_a = np.asarray(bq, dtype=np.float32)
    bv_a = np.asarray(bv, dtype=np.float32)

    bf = ml_dtypes.bfloat16

    # per 128-row contraction tile [128, kt, 128]: [Wk|Wv]
    def pack2(wa, wb):
        h = np.empty((128, NKT, 128), np.float32)
        h[:, :, :64] = wa.reshape(NKT, 128, 64).transpose(1, 0, 2)
        h[:, :, 64:] = wb.reshape(NKT, 128, 64).transpose(1, 0, 2)
        return np.ascontiguousarray(h.reshape(128, NKT * 128)).astype(bf)

    wkv_e_host = pack2(Wk, Wv).reshape(128, NKT, 128)
    wq2_host = pack2(Wq, Wq).reshape(128, NKT, 128)
    bias_host = np.zeros((128, 1), np.float32)
    bias_host[:, 0] = np.concatenate([bq_a, bq_a])
    ident_host = np.eye(128, dtype=np.float32)

    in_maps = []
    for c in range(NCORES):
        b, h = divmod(c, 2)
        xb = x[b]
        if h == 1:
            xb = np.concatenate([xb[TQ:], xb[:TQ]], axis=0)
        # xH[p, c, k, t] = x^T[k*128+p, c*512+t]
        xh = np.ascontiguousarray(
            xb.T.astype(bf).reshape(NKT, 128, NCH, CH).transpose(1, 2, 0, 3)
        ).reshape(128, NCH, NKT, CH)
        in_maps.append(
            {
                "xH": xh,
                "wkv_e": wkv_e_host,
                "wq2": wq2_host,
                "biasd": bias_host,
                "identd": ident_host,
            }
        )

    nc = _get_nc()
    res = run_bass_kernel_spmd(nc, in_maps, core_ids=list(range(NCORES)), **RUN_KWARGS)
    LAST_RESULT = res

    outp = np.empty((B, S, H), np.float32)
    for c in range(NCORES):
        b, h = divmod(c, 2)
        on = np.asarray(res.results[c]["outn"], np.float32)  # [qs, 128, 512]
        num = on[:, 0:64, :] + on[:, 64:128, :]     # [qs, h, q]
        den = res.results[c]["outd"].sum(axis=1, keepdims=True)
        outp[b, h * TQ : (h + 1) * TQ] = (
            (num / den).transpose(0, 2, 1).reshape(TQ, H) + bv_a
        )
    return outp
